# revision 1
# baseline (speedup 1.0000x reference)
"""CapsuleLayer (dynamic routing) Trainium2 kernel.

Full inputs -> batch-sharded over 8 NeuronCores -> full output.

Math (per sample b):
    ihat[i,c,o] = sum_d x[i,d] * W[i,c,d,o]
    bias = 0
    for r in 0..2:
        coup = softmax(bias, axis=c)
        s[c,o] = sum_i coup[i,c] * ihat[i,c,o]
        v = squash(s)
        if r < 2: bias[i,c] += sum_o ihat[i,c,o] * v[c,o]
    return v

Device layout (per core, 32 local samples, batch-tiles of 8):
    SBUF partition dim p = (b, i_sub): p = b*16 + i_sub   (b in 0..7 of tile,
    i_sub = i % 16), free dim (ig, c, o) with ig = i // 16 (72 groups).
    ihat tile: [128, 72*10*16]

    einsum: per (bt, ig) one matmul, lhsT = host-prepared block-diagonal
    x tile [ (i_sub,d)=128, (b,i_sub')=128 ], rhs = W chunk [128, 160].

    routing weighted sum: per ig matmul with lhsT = coupZ [128, (c',b')=80]
    (coup placed on the b'=b diagonal via a mask multiply), PSUM-accumulated
    over ig. The [80,160] result holds s[b,c,:] in its (c'==c) diagonal
    blocks; it is masked on evacuation, squashed with per-partition scalars,
    and collapsed to [8,160] with a selector matmul (engine partition ranges
    must start 32-aligned, so no sub-32 partition slicing anywhere).
"""

import sys

if "/opt/trn_rl_repo" not in sys.path:
    sys.path.insert(0, "/opt/trn_rl_repo")

import numpy as np

B, I, D, C, O = 256, 1152, 8, 10, 16
NCORES = 8
BL = B // NCORES            # 32 local samples per core
NBT, BT = 4, 8              # batch tiles
ISUB = 16                   # i's per group
IG = I // ISUB              # 72 groups
CO = C * O                  # 160
NR = 3
EPS = 1e-7
XZ_CHUNK = 18               # ig's per xz DMA chunk
F32 = np.float32

# bf16 for PE-heavy tensors (einsum inputs, ihat storage, coupling weights);
# routing state (bias, coup, softmax, squash, v) stays fp32.
USE_BF16 = True

_compiled = {}


def _build_program():
    import concourse.bacc as bacc
    import concourse.tile as tile
    import concourse.mybir as mybir
    import concourse.bass as bass

    f32 = mybir.dt.float32
    lo = mybir.dt.bfloat16 if USE_BF16 else f32
    nc = bacc.Bacc("TRN2", target_bir_lowering=False, debug=False,
                   num_devices=NCORES)

    xz_t = nc.dram_tensor("xz", [NBT * IG, 128, 128], lo, kind="ExternalInput")
    xt_t = nc.dram_tensor("xt", [128, IG, BL], lo, kind="ExternalInput")
    w_t = nc.dram_tensor("w", [128, IG * CO], lo, kind="ExternalInput")
    cmask_t = nc.dram_tensor("cmask", [C * BT, CO], f32, kind="ExternalInput")
    maskz_t = nc.dram_tensor("maskz", [128, C * BT], lo, kind="ExternalInput")
    sel_t = nc.dram_tensor("sel", [C * BT, BT], f32, kind="ExternalInput")
    out_t = nc.dram_tensor("out", [BL, CO], f32, kind="ExternalOutput")
    vscr_t = nc.dram_tensor("vscr", [BL, CO], f32)   # internal scratch
    xz_ap, xt_ap, w_ap = xz_t.ap(), xt_t.ap(), w_t.ap()
    out_ap, vscr_ap = out_t.ap(), vscr_t.ap()

    AF = mybir.ActivationFunctionType
    ALU = mybir.AluOpType
    AX = mybir.AxisListType

    with tile.TileContext(nc) as tc:
        from contextlib import ExitStack

        with ExitStack() as ctx:
            singles = ctx.enter_context(tc.tile_pool(name="singles", bufs=1))
            xzp = ctx.enter_context(tc.tile_pool(name="xzp", bufs=3))
            psum = ctx.enter_context(tc.tile_pool(name="psum", bufs=4, space="PSUM"))
            psm = ctx.enter_context(tc.tile_pool(name="psm", bufs=1, space="PSUM"))
            tch = ctx.enter_context(tc.tile_pool(name="tch", bufs=2))
            sm = ctx.enter_context(tc.tile_pool(name="sm", bufs=2))

            w_sb = singles.tile([128, IG * CO], lo)
            nc.sync.dma_start(out=w_sb, in_=w_ap)
            xt_sb = singles.tile([128, IG * BL], lo)
            nc.sync.dma_start(out=xt_sb,
                              in_=xt_ap.rearrange("p g b -> p (g b)"))
            cmask = singles.tile([C * BT, CO], f32)
            nc.sync.dma_start(out=cmask, in_=cmask_t.ap())
            maskz = singles.tile([128, C * BT], lo)
            nc.sync.dma_start(out=maskz, in_=maskz_t.ap())
            sel_sb = singles.tile([C * BT, BT], f32)
            nc.sync.dma_start(out=sel_sb, in_=sel_t.ap())

            ihp = ctx.enter_context(tc.tile_pool(name="ihp", bufs=2))
            zsc = singles.tile([128, IG * C * BT], lo)      # coupZ
            bias = singles.tile([128, IG * C], f32)
            tmp720 = singles.tile([128, IG * C], f32)
            coup = singles.tile([128, IG * C], f32)
            zsum = singles.tile([128, IG], f32)
            vrep = singles.tile([128, CO], lo)

            # ---- r0 weighted sum: s0 = 0.1 * sum_{i,d} x*W  (all 32 b) ----
            ps0 = psm.tile([BL, CO], f32)
            for kc in range(IG):
                nc.tensor.matmul(ps0, xt_sb[:, kc * BL:(kc + 1) * BL],
                                 w_sb[:, kc * CO:(kc + 1) * CO],
                                 start=(kc == 0), stop=(kc == IG - 1))
            s_all = singles.tile([BL, CO], f32)
            nc.scalar.mul(s_all, ps0, 1.0 / C)

            # ---- squash32: reference squash on a [32, (c,o)] tile --------
            def nr_rsqrt(pool, a, p, w):
                """exact-ish rsqrt(a) via Sqrt table seed + 2 Newton steps"""
                sq = pool.tile([p, w], f32)
                nc.scalar.activation(sq, a, AF.Sqrt)
                rs = pool.tile([p, w], f32)
                nc.vector.reciprocal(rs, sq)
                t1 = pool.tile([p, w], f32)
                t2 = pool.tile([p, w], f32)
                for _ in range(2):
                    nc.vector.tensor_mul(t1, a, rs)
                    nc.vector.tensor_mul(t1, t1, rs)
                    nc.vector.tensor_scalar(t2, t1, -0.5, 1.5,
                                            op0=ALU.mult, op1=ALU.add)
                    nc.vector.tensor_mul(rs, rs, t2)
                return rs

            def squash_factor(pool, n2, p, w):
                """f = n2 / ((1+n2) * sqrt(n2+eps)), elementwise [p, w]"""
                a = pool.tile([p, w], f32)
                nc.vector.tensor_scalar_add(a, n2, EPS)
                rs = nr_rsqrt(pool, a, p, w)
                dn = pool.tile([p, w], f32)
                nc.vector.tensor_scalar_add(dn, n2, 1.0)
                di = pool.tile([p, w], f32)
                nc.vector.reciprocal(di, dn)
                f = pool.tile([p, w], f32)
                nc.vector.tensor_mul(f, n2, rs)
                nc.vector.tensor_mul(f, f, di)
                return f

            # r0 squash on [32, CO]
            sq32 = singles.tile([BL, CO], f32)
            nc.vector.tensor_mul(sq32, s_all, s_all)
            n2_32 = singles.tile([BL, C], f32)
            nc.vector.tensor_reduce(
                n2_32, sq32.rearrange("p (c o) -> p c o", c=C),
                axis=AX.X, op=ALU.add)
            f32t = squash_factor(singles, n2_32, BL, C)
            v0 = singles.tile([BL, CO], f32)
            fb = bass.AP(tensor=f32t.tensor, offset=f32t.offset,
                         ap=[f32t.ap[0], f32t.ap[1], [0, O]])
            nc.vector.tensor_tensor(v0, s_all, fb, op=ALU.mult)
            nc.sync.dma_start(out=vscr_ap, in_=v0)

            for bt in range(NBT):
                # ================= einsum: ihat for this batch tile =========
                ihat = ihp.tile([128, IG * CO], lo)
                for ch in range(IG // XZ_CHUNK):
                    xz_sb = xzp.tile([128, XZ_CHUNK * 128], lo)
                    base = bt * IG + ch * XZ_CHUNK
                    nc.sync.dma_start(
                        out=xz_sb.rearrange("p (t m) -> p t m", t=XZ_CHUNK),
                        in_=xz_ap[base:base + XZ_CHUNK].rearrange(
                            "t p m -> p t m"))
                    for t in range(XZ_CHUNK):
                        ig = ch * XZ_CHUNK + t
                        pih = psum.tile([128, CO], f32)
                        nc.tensor.matmul(pih, xz_sb[:, t * 128:(t + 1) * 128],
                                         w_sb[:, ig * CO:(ig + 1) * CO],
                                         start=True, stop=True)
                        dst = ihat[:, ig * CO:(ig + 1) * CO]
                        if ig % 2 == 0:
                            nc.vector.tensor_copy(dst, pih)
                        else:
                            nc.scalar.copy(dst, pih)

                vsrc = None   # None -> use vscr dram rows for this bt (r0)
                for r in range(NR - 1):
                    # ---- vrep[p=(b,i_sub), co] = v[b, co] ------------------
                    if vsrc is None:
                        vi = bass.AP(tensor=vscr_ap.tensor,
                                     offset=bt * BT * CO,
                                     ap=[[CO, BT], [0, ISUB], [1, CO]])
                    else:
                        vi = bass.AP(tensor=vsrc.tensor, offset=vsrc.offset,
                                     ap=[vsrc.ap[0], [0, ISUB], [1, CO]])
                    nc.gpsimd.dma_start(out=vrep, in_=vi)
                    # ---- bias (+)= sum_o ihat * vrep -----------------------
                    for ch in range(4):
                        g0 = ch * (IG // 4)
                        gn = IG // 4
                        tc_t = tch.tile([128, gn * CO], lo)
                        vb = bass.AP(tensor=vrep.tensor, offset=vrep.offset,
                                     ap=[vrep.ap[0], [0, gn], [1, CO]])
                        nc.vector.tensor_tensor(
                            tc_t, ihat[:, g0 * CO:(g0 + gn) * CO], vb,
                            op=ALU.mult)
                        red_dst = (bias if r == 0 else tmp720)[
                            :, g0 * C:(g0 + gn) * C]
                        nc.vector.tensor_reduce(
                            red_dst,
                            tc_t.rearrange("p (gc o) -> p gc o", o=O),
                            axis=AX.X, op=ALU.add)
                    if r > 0:
                        nc.vector.tensor_add(bias, bias, tmp720)

                    # ---- coup = softmax(bias) over c -----------------------
                    nc.scalar.activation(coup, bias, AF.Exp)
                    nc.vector.tensor_reduce(
                        zsum, coup.rearrange("p (g c) -> p g c", c=C),
                        axis=AX.X, op=ALU.add)
                    rz = sm.tile([128, IG], f32)
                    nc.vector.reciprocal(rz, zsum)
                    rzb = bass.AP(tensor=rz.tensor, offset=rz.offset,
                                  ap=[rz.ap[0], rz.ap[1], [0, C]])
                    nc.vector.tensor_tensor(coup, coup, rzb, op=ALU.mult)

                    # ---- zsc[(b,i),(g,c,b')] = coup[(b,i),(g,c)]*d(b,b') ---
                    zr = zsc.rearrange("p (g c b) -> p g c b", c=C, b=BT)
                    cr = coup.rearrange("p (g c) -> p g c", c=C)
                    cb = bass.AP(tensor=cr.tensor, offset=cr.offset,
                                 ap=[cr.ap[0], cr.ap[1], cr.ap[2], [0, BT]])
                    mr = maskz.rearrange("p (c b) -> p c b", b=BT)
                    mb = bass.AP(tensor=mr.tensor, offset=mr.offset,
                                 ap=[mr.ap[0], [0, IG], mr.ap[1], mr.ap[2]])
                    nc.vector.tensor_tensor(zr, cb, mb, op=ALU.mult)

                    # ---- s = sum_i coup*ihat via PE ------------------------
                    pss = psm.tile([C * BT, CO], f32)
                    for ig in range(IG):
                        nc.tensor.matmul(
                            pss, zsc[:, ig * C * BT:(ig + 1) * C * BT],
                            ihat[:, ig * CO:(ig + 1) * CO],
                            start=(ig == 0), stop=(ig == IG - 1))
                    # masked evacuation: sst[(c',b),(c,o)] = pss * d(c,c')
                    sst = sm.tile([C * BT, CO], f32)
                    nc.vector.tensor_tensor(sst, pss, cmask, op=ALU.mult)
                    # n2 per partition (c',b):  sum over free of sst^2
                    sjunk = sm.tile([C * BT, CO], f32)
                    n2_80 = sm.tile([C * BT, 1], f32)
                    nc.vector.scalar_tensor_tensor(
                        sjunk, sst, 1.0, sst, op0=ALU.mult, op1=ALU.mult,
                        accum_out=n2_80)
                    f80 = squash_factor(sm, n2_80, C * BT, 1)
                    v80 = sm.tile([C * BT, CO], f32)
                    nc.vector.tensor_scalar_mul(v80, sst, f80)
                    # collapse (c',b) -> b with selector matmul
                    v8ps = psm.tile([BT, CO], f32)
                    nc.tensor.matmul(v8ps, sel_sb, v80, start=True, stop=True)
                    v_sb = sm.tile([BT, CO], f32)
                    nc.vector.tensor_copy(v_sb, v8ps)
                    vsrc = v_sb

                nc.sync.dma_start(out=out_ap[bt * BT:(bt + 1) * BT, :],
                                  in_=vsrc)

    nc.compile()
    return nc


def _prep_inputs(x, W):
    """Host-side layout transforms (not part of measured HW time)."""
    x = np.ascontiguousarray(x, dtype=F32)
    W = np.ascontiguousarray(W, dtype=F32)
    # W -> [(i_sub, d), (ig, c, o)]
    wr = np.ascontiguousarray(
        W.reshape(IG, ISUB, C, D, O).transpose(1, 3, 0, 2, 4)
    ).reshape(128, IG * CO)

    # x -> per core [core, bt, b, ig, i_sub, d]
    x8 = x.reshape(NCORES, NBT, BT, IG, ISUB, D)

    # block-diagonal lhsT tiles: xz[core, bt, ig, (i_sub,d), (b,i_sub')]
    xz = np.zeros((NCORES, NBT, IG, ISUB, D, 128), dtype=F32)
    isub = np.arange(ISUB)
    for b in range(BT):
        # advanced indexing pulls the i_sub axis to the front
        xz[:, :, :, isub, :, b * ISUB + isub] = \
            x8[:, :, b].transpose(3, 0, 1, 2, 4)
    xz = xz.reshape(NCORES, NBT * IG, 128, 128)

    # compact xT for r0: [core, (i_sub,d), ig, b]
    xt = np.ascontiguousarray(
        x8.reshape(NCORES, BL, IG, ISUB, D).transpose(0, 3, 4, 2, 1)
    ).reshape(NCORES, 128, IG, BL)

    # constants
    cmask = np.zeros((C * BT, CO), dtype=F32)       # [(c',b), (c,o)]
    for c in range(C):
        cmask[c * BT:(c + 1) * BT, c * O:(c + 1) * O] = 1.0
    # maskz[p=(b,i), (c,b')] = 1 iff b' == b
    maskz = np.zeros((128, C * BT), dtype=F32)      # [(b,i_sub), (c,b')]
    for b in range(BT):
        for c in range(C):
            maskz[b * ISUB:(b + 1) * ISUB, c * BT + b] = 1.0
    sel = np.zeros((C * BT, BT), dtype=F32)         # [(c',b), b']
    for c in range(C):
        for b in range(BT):
            sel[c * BT + b, b] = 1.0

    if USE_BF16:
        from ml_dtypes import bfloat16
        xz = xz.astype(bfloat16)
        xt = xt.astype(bfloat16)
        wr = wr.astype(bfloat16)
        maskz = maskz.astype(bfloat16)
    return xz, xt, wr, cmask, maskz, sel


def kernel(x: np.ndarray, W: np.ndarray) -> np.ndarray:
    from concourse import bass_utils

    if "nc" not in _compiled:
        _compiled["nc"] = _build_program()
    nc = _compiled["nc"]

    xz, xt, wr, cmask, maskz, sel = _prep_inputs(np.asarray(x), np.asarray(W))
    in_maps = [{"xz": xz[c], "xt": xt[c], "w": wr,
                "cmask": cmask, "maskz": maskz, "sel": sel}
               for c in range(NCORES)]
    res = bass_utils.run_bass_kernel_spmd(nc, in_maps, list(range(NCORES)))
    out = np.concatenate([res.results[c]["out"] for c in range(NCORES)], axis=0)
    return out.reshape(B, C, O)



# revision 3
# speedup vs baseline: 1.1790x; 1.1790x over previous
"""CapsuleLayer (dynamic routing) Trainium2 kernel — v2.

Full inputs -> batch-sharded over 8 NeuronCores -> full output.

Math (per sample b):
    ihat[i,c,o] = sum_d x[i,d] * W[i,c,d,o]
    bias = 0
    for r in 0..2:
        coup = softmax(bias, axis=c)
        s[c,o] = sum_i coup[i,c] * ihat[i,c,o]
        v = squash(s)
        if r < 2: bias[i,c] += sum_o ihat[i,c,o] * v[c,o]
    return v

Device layout (per core, 32 local samples, batch-tiles of 8):
    SBUF partition dim p = (b, i_sub): p = b*16 + i_sub, free dim (ig, c, o)
    with ig = i // 16 (72 groups).  ihat tile: [128, 72*10*16] bf16.

v2 perf notes (vs v1, all DVE-bottleneck driven):
  - o-reduction of ihat*v uses bf16 2x-mode pairwise tree adds instead of
    1x-only tensor_reduce.
  - softmax state kept multiplicatively: e *= exp(delta) instead of
    f32 bias accumulation; 1/z and rsqrt via Ln/Exp on the scalar engine
    so the whole kernel uses ONE activation table set
    (natural_log_exp_and_others: exp/ln/square/copy).
  - zsc coupling lhsT layout (g, b, c) with all-bf16 step-1 operands ->
    2x-mode mask multiply.
  - einsum PSUM evacuation batched 3 ig per 2KB PSUM bank, single
    scalar-engine copy each (PE matmuls write [128,480] banks).
"""

import sys

if "/opt/trn_rl_repo" not in sys.path:
    sys.path.insert(0, "/opt/trn_rl_repo")

import numpy as np

B, I, D, C, O = 256, 1152, 8, 10, 16
NCORES = 8
BL = B // NCORES            # 32 local samples per core
NBT, BT = 4, 8              # batch tiles
ISUB = 16                   # i's per group
IG = I // ISUB              # 72 groups
CO = C * O                  # 160
NR = 3
EPS = 1e-7
XZ_CHUNK = 18               # ig's per xz DMA chunk
NCH = 4                     # bias-update chunks (IG/NCH groups each)
F32 = np.float32

USE_BF16 = True

_compiled = {}


def _build_program():
    import concourse.bacc as bacc
    import concourse.tile as tile
    import concourse.mybir as mybir
    import concourse.bass as bass

    f32 = mybir.dt.float32
    lo = mybir.dt.bfloat16 if USE_BF16 else f32
    nc = bacc.Bacc("TRN2", target_bir_lowering=False, debug=False,
                   num_devices=NCORES)

    xz_t = nc.dram_tensor("xz", [NBT * IG, 128, 128], lo, kind="ExternalInput")
    xt_t = nc.dram_tensor("xt", [128, IG, BL], lo, kind="ExternalInput")
    w_t = nc.dram_tensor("w", [128, IG * CO], lo, kind="ExternalInput")
    cmask_t = nc.dram_tensor("cmask", [BT * C, CO], f32, kind="ExternalInput")
    maskz_t = nc.dram_tensor("maskz", [128, BT * C], lo, kind="ExternalInput")
    sel_t = nc.dram_tensor("sel", [BT * C, BT], f32, kind="ExternalInput")
    out_t = nc.dram_tensor("out", [BL, CO], f32, kind="ExternalOutput")
    vscr_t = nc.dram_tensor("vscr", [BL, CO], f32)   # internal scratch
    xz_ap, xt_ap, w_ap = xz_t.ap(), xt_t.ap(), w_t.ap()
    out_ap, vscr_ap = out_t.ap(), vscr_t.ap()

    AF = mybir.ActivationFunctionType
    ALU = mybir.AluOpType
    AX = mybir.AxisListType

    GN = IG // NCH           # 18 groups per bias-update chunk
    GC = GN * C              # 180 (g,c) pairs per chunk

    with tile.TileContext(nc) as tc:
        from contextlib import ExitStack

        with ExitStack() as ctx:
            singles = ctx.enter_context(tc.tile_pool(name="singles", bufs=1))
            xzp = ctx.enter_context(tc.tile_pool(name="xzp", bufs=3))
            psum = ctx.enter_context(tc.tile_pool(name="psum", bufs=4, space="PSUM"))
            psm = ctx.enter_context(tc.tile_pool(name="psm", bufs=1, space="PSUM"))
            tch = ctx.enter_context(tc.tile_pool(name="tch", bufs=2))
            trp = ctx.enter_context(tc.tile_pool(name="trp", bufs=2))
            sm = ctx.enter_context(tc.tile_pool(name="sm", bufs=2))

            w_sb = singles.tile([128, IG * CO], lo)
            nc.sync.dma_start(out=w_sb, in_=w_ap)
            xt_sb = singles.tile([128, IG * BL], lo)
            nc.sync.dma_start(out=xt_sb,
                              in_=xt_ap.rearrange("p g b -> p (g b)"))
            cmask = singles.tile([BT * C, CO], f32)
            nc.sync.dma_start(out=cmask, in_=cmask_t.ap())
            maskz = singles.tile([128, BT * C], lo)
            nc.sync.dma_start(out=maskz, in_=maskz_t.ap())
            sel_sb = singles.tile([BT * C, BT], f32)
            nc.sync.dma_start(out=sel_sb, in_=sel_t.ap())

            ihp = ctx.enter_context(tc.tile_pool(name="ihp", bufs=2))
            zsc = singles.tile([128, IG * BT * C], lo)   # coupling lhsT
            e_t = singles.tile([128, IG * C], lo)        # exp(bias) running
            delta = singles.tile([128, IG * C], f32)     # sum_o ihat*v
            coup = singles.tile([128, IG * C], lo)
            zsum = singles.tile([128, IG], f32)
            lnz = singles.tile([128, IG], f32)
            rz = singles.tile([128, IG], lo)
            vrep = singles.tile([128, CO], lo)

            # ---- r0 weighted sum: s0 = 0.1 * sum_{i,d} x*W  (all 32 b) ----
            ps0 = psm.tile([BL, CO], f32)
            for kc in range(IG):
                nc.tensor.matmul(ps0, xt_sb[:, kc * BL:(kc + 1) * BL],
                                 w_sb[:, kc * CO:(kc + 1) * CO],
                                 start=(kc == 0), stop=(kc == IG - 1))
            s_all = singles.tile([BL, CO], f32)
            nc.scalar.mul(s_all, ps0, 1.0 / C)

            epsc = singles.tile([128, 1], f32)
            nc.gpsimd.memset(epsc, EPS)

            # squash factor f = n2 * exp(-ln(1+n2) - 0.5*ln(n2+eps))
            # (rsqrt and reciprocal via Ln/Exp: same activation table set)
            def squash_factor(pool, n2, p, w):
                l1 = pool.tile([p, w], f32)
                nc.scalar.activation(l1, n2, AF.Ln, bias=1.0)
                l2 = pool.tile([p, w], f32)
                nc.scalar.activation(l2, n2, AF.Ln, bias=epsc[:p])
                u = pool.tile([p, w], f32)
                nc.vector.scalar_tensor_tensor(u, l2, -0.5, l1,
                                               op0=ALU.mult, op1=ALU.subtract)
                g = pool.tile([p, w], f32)
                nc.scalar.activation(g, u, AF.Exp)
                f = pool.tile([p, w], f32)
                nc.vector.tensor_mul(f, n2, g)
                return f

            # r0 squash on [32, CO]
            sq32 = singles.tile([BL, CO], f32)
            nc.vector.tensor_mul(sq32, s_all, s_all)
            n2_32 = singles.tile([BL, C], f32)
            nc.vector.tensor_reduce(
                n2_32, sq32.rearrange("p (c o) -> p c o", c=C),
                axis=AX.X, op=ALU.add)
            f32t = squash_factor(singles, n2_32, BL, C)
            v0 = singles.tile([BL, CO], f32)
            fb = bass.AP(tensor=f32t.tensor, offset=f32t.offset,
                         ap=[f32t.ap[0], f32t.ap[1], [0, O]])
            nc.vector.tensor_tensor(v0, s_all, fb, op=ALU.mult)
            nc.sync.dma_start(out=vscr_ap, in_=v0)

            for bt in range(NBT):
                # ================= einsum: ihat for this batch tile =========
                ihat = ihp.tile([128, IG * CO], lo)
                for ch in range(IG // XZ_CHUNK):
                    xz_sb = xzp.tile([128, XZ_CHUNK * 128], lo)
                    base = bt * IG + ch * XZ_CHUNK
                    nc.sync.dma_start(
                        out=xz_sb.rearrange("p (t m) -> p t m", t=XZ_CHUNK),
                        in_=xz_ap[base:base + XZ_CHUNK].rearrange(
                            "t p m -> p t m"))
                    for t3 in range(XZ_CHUNK // 3):
                        pih = psum.tile([128, 3 * CO], f32)
                        for j in range(3):
                            t = t3 * 3 + j
                            ig = ch * XZ_CHUNK + t
                            nc.tensor.matmul(
                                pih[:, j * CO:(j + 1) * CO],
                                xz_sb[:, t * 128:(t + 1) * 128],
                                w_sb[:, ig * CO:(ig + 1) * CO],
                                start=True, stop=True)
                        ig0 = ch * XZ_CHUNK + t3 * 3
                        nc.scalar.copy(
                            ihat[:, ig0 * CO:(ig0 + 3) * CO], pih)

                vsrc = None   # None -> use vscr dram rows for this bt (r0)
                for r in range(NR - 1):
                    # ---- vrep[p=(b,i_sub), co] = v[b, co] ------------------
                    if vsrc is None:
                        vi = bass.AP(tensor=vscr_ap.tensor,
                                     offset=bt * BT * CO,
                                     ap=[[CO, BT], [0, ISUB], [1, CO]])
                    else:
                        vi = bass.AP(tensor=vsrc.tensor, offset=vsrc.offset,
                                     ap=[vsrc.ap[0], [0, ISUB], [1, CO]])
                    nc.gpsimd.dma_start(out=vrep, in_=vi)

                    # ---- delta[p,(g,c)] = sum_o ihat*vrep (2x tree) --------
                    for ch in range(NCH):
                        g0 = ch * GN
                        tc_t = tch.tile([128, GN * CO], lo)
                        vb = bass.AP(tensor=vrep.tensor, offset=vrep.offset,
                                     ap=[vrep.ap[0], [0, GN], [1, CO]])
                        nc.vector.tensor_tensor(
                            tc_t, ihat[:, g0 * CO:(g0 + GN) * CO], vb,
                            op=ALU.mult)
                        t8 = trp.tile([128, GC * 8], lo)
                        a0 = bass.AP(tensor=tc_t.tensor, offset=tc_t.offset,
                                     ap=[tc_t.ap[0], [16, GC], [1, 8]])
                        a1 = bass.AP(tensor=tc_t.tensor,
                                     offset=tc_t.offset + 8,
                                     ap=[tc_t.ap[0], [16, GC], [1, 8]])
                        d8 = bass.AP(tensor=t8.tensor, offset=t8.offset,
                                     ap=[t8.ap[0], [8, GC], [1, 8]])
                        nc.vector.tensor_tensor(d8, a0, a1, op=ALU.add)
                        t4 = trp.tile([128, GC * 4], lo)
                        b0 = bass.AP(tensor=t8.tensor, offset=t8.offset,
                                     ap=[t8.ap[0], [8, GC], [1, 4]])
                        b1 = bass.AP(tensor=t8.tensor, offset=t8.offset + 4,
                                     ap=[t8.ap[0], [8, GC], [1, 4]])
                        d4 = bass.AP(tensor=t4.tensor, offset=t4.offset,
                                     ap=[t4.ap[0], [4, GC], [1, 4]])
                        nc.vector.tensor_tensor(d4, b0, b1, op=ALU.add)
                        t2 = trp.tile([128, GC * 2], lo)
                        c0 = bass.AP(tensor=t4.tensor, offset=t4.offset,
                                     ap=[t4.ap[0], [4, GC], [1, 2]])
                        c1 = bass.AP(tensor=t4.tensor, offset=t4.offset + 2,
                                     ap=[t4.ap[0], [4, GC], [1, 2]])
                        d2 = bass.AP(tensor=t2.tensor, offset=t2.offset,
                                     ap=[t2.ap[0], [2, GC], [1, 2]])
                        nc.vector.tensor_tensor(d2, c0, c1, op=ALU.add)
                        e0 = bass.AP(tensor=t2.tensor, offset=t2.offset,
                                     ap=[t2.ap[0], [2, GC]])
                        e1a = bass.AP(tensor=t2.tensor, offset=t2.offset + 1,
                                      ap=[t2.ap[0], [2, GC]])
                        nc.vector.tensor_tensor(
                            delta[:, ch * GC:(ch + 1) * GC], e0, e1a,
                            op=ALU.add)

                    # ---- e = exp(bias): multiplicative accumulation --------
                    if r == 0:
                        nc.scalar.activation(e_t, delta, AF.Exp)
                    else:
                        ed = sm.tile([128, IG * C], lo)
                        nc.scalar.activation(ed, delta, AF.Exp)
                        nc.vector.tensor_tensor(e_t, e_t, ed, op=ALU.mult)

                    # ---- coup = e / z ------------------------------------
                    nc.vector.tensor_reduce(
                        zsum, e_t.rearrange("p (g c) -> p g c", c=C),
                        axis=AX.X, op=ALU.add)
                    nc.scalar.activation(lnz, zsum, AF.Ln)
                    nc.scalar.activation(rz, lnz, AF.Exp, scale=-1.0)
                    rzb = bass.AP(tensor=rz.tensor, offset=rz.offset,
                                  ap=[rz.ap[0], rz.ap[1], [0, C]])
                    nc.vector.tensor_tensor(coup, e_t, rzb, op=ALU.mult)

                    # ---- zsc[(b,i),(g,b',c)] = coup[(b,i),(g,c)]*d(b,b') ---
                    zr = zsc.rearrange("p (g b c) -> p g b c", b=BT, c=C)
                    cb = bass.AP(tensor=coup.tensor, offset=coup.offset,
                                 ap=[coup.ap[0], [C, IG], [0, BT], [1, C]])
                    mb = bass.AP(tensor=maskz.tensor, offset=maskz.offset,
                                 ap=[maskz.ap[0], [0, IG], [C, BT], [1, C]])
                    nc.vector.tensor_tensor(zr, cb, mb, op=ALU.mult)

                    # ---- s = sum_i coup*ihat via PE ------------------------
                    pss = psm.tile([BT * C, CO], f32)
                    for ig in range(IG):
                        nc.tensor.matmul(
                            pss, zsc[:, ig * BT * C:(ig + 1) * BT * C],
                            ihat[:, ig * CO:(ig + 1) * CO],
                            start=(ig == 0), stop=(ig == IG - 1))
                    # masked evacuation: sst[(b,c'),(c,o)] = pss * d(c,c')
                    sst = sm.tile([BT * C, CO], f32)
                    nc.vector.tensor_tensor(sst, pss, cmask, op=ALU.mult)
                    # n2 per partition (b,c'): sum over free of sst^2
                    sjunk = sm.tile([BT * C, CO], f32)
                    n2_80 = sm.tile([BT * C, 1], f32)
                    nc.scalar.activation(sjunk, sst, AF.Square,
                                         accum_out=n2_80)
                    f80 = squash_factor(sm, n2_80, BT * C, 1)
                    v80 = sm.tile([BT * C, CO], f32)
                    nc.vector.tensor_scalar_mul(v80, sst, f80)
                    # collapse (b,c') -> b with selector matmul
                    v8ps = psm.tile([BT, CO], f32)
                    nc.tensor.matmul(v8ps, sel_sb, v80, start=True, stop=True)
                    v_sb = sm.tile([BT, CO], f32)
                    nc.vector.tensor_copy(v_sb, v8ps)
                    vsrc = v_sb

                nc.sync.dma_start(out=out_ap[bt * BT:(bt + 1) * BT, :],
                                  in_=vsrc)

    nc.compile()
    return nc


def _prep_inputs(x, W):
    """Host-side layout transforms (not part of measured HW time)."""
    x = np.ascontiguousarray(x, dtype=F32)
    W = np.ascontiguousarray(W, dtype=F32)
    # W -> [(i_sub, d), (ig, c, o)]
    wr = np.ascontiguousarray(
        W.reshape(IG, ISUB, C, D, O).transpose(1, 3, 0, 2, 4)
    ).reshape(128, IG * CO)

    # x -> per core [core, bt, b, ig, i_sub, d]
    x8 = x.reshape(NCORES, NBT, BT, IG, ISUB, D)

    # block-diagonal lhsT tiles: xz[core, bt, ig, (i_sub,d), (b,i_sub')]
    xz = np.zeros((NCORES, NBT, IG, ISUB, D, 128), dtype=F32)
    isub = np.arange(ISUB)
    for b in range(BT):
        # advanced indexing pulls the i_sub axis to the front
        xz[:, :, :, isub, :, b * ISUB + isub] = \
            x8[:, :, b].transpose(3, 0, 1, 2, 4)
    xz = xz.reshape(NCORES, NBT * IG, 128, 128)

    # compact xT for r0: [core, (i_sub,d), ig, b]
    xt = np.ascontiguousarray(
        x8.reshape(NCORES, BL, IG, ISUB, D).transpose(0, 3, 4, 2, 1)
    ).reshape(NCORES, 128, IG, BL)

    # constants (all (b,c)-ordered partition/row layouts)
    cmask = np.zeros((BT * C, CO), dtype=F32)       # [(b,c'), (c,o)]
    for b in range(BT):
        for c in range(C):
            cmask[b * C + c, c * O:(c + 1) * O] = 1.0
    # maskz[p=(b,i), (b',c)] = 1 iff b' == b
    maskz = np.zeros((128, BT * C), dtype=F32)
    for b in range(BT):
        for c in range(C):
            maskz[b * ISUB:(b + 1) * ISUB, b * C + c] = 1.0
    sel = np.zeros((BT * C, BT), dtype=F32)         # [(b,c'), b2]
    for b in range(BT):
        for c in range(C):
            sel[b * C + c, b] = 1.0

    if USE_BF16:
        from ml_dtypes import bfloat16
        xz = xz.astype(bfloat16)
        xt = xt.astype(bfloat16)
        wr = wr.astype(bfloat16)
        maskz = maskz.astype(bfloat16)
    return xz, xt, wr, cmask, maskz, sel


def kernel(x: np.ndarray, W: np.ndarray) -> np.ndarray:
    from concourse import bass_utils

    if "nc" not in _compiled:
        _compiled["nc"] = _build_program()
    nc = _compiled["nc"]

    xz, xt, wr, cmask, maskz, sel = _prep_inputs(np.asarray(x), np.asarray(W))
    in_maps = [{"xz": xz[c], "xt": xt[c], "w": wr,
                "cmask": cmask, "maskz": maskz, "sel": sel}
               for c in range(NCORES)]
    res = bass_utils.run_bass_kernel_spmd(nc, in_maps, list(range(NCORES)))
    out = np.concatenate([res.results[c]["out"] for c in range(NCORES)], axis=0)
    return out.reshape(B, C, O)


# revision 7
# speedup vs baseline: 1.6019x; 1.3587x over previous
"""CapsuleLayer (dynamic routing) Trainium2 kernel — v3.

Full inputs -> batch-sharded over 8 NeuronCores -> full output.

Math (per sample b):
    ihat[i,c,o] = sum_d x[i,d] * W[i,c,d,o]
    bias = 0
    for r in 0..2:
        coup = softmax(bias, axis=c)
        s[c,o] = sum_i coup[i,c] * ihat[i,c,o]
        v = squash(s)
        if r < 2: bias[i,c] += sum_o ihat[i,c,o] * v[c,o]
    return v

Device layout (per core, 32 local samples, batch-tiles of 8):
    SBUF partition dim p = (b, i_sub): p = b*16 + i_sub, free dim (ig, c, o)
    with ig = i // 16 (72 groups).  ihat tile: [128, 72*10*16] bf16.

v3 perf notes:
  - Exp is the ONLY table-based scalar activation (one ACT_TABLE_LOAD
    total).  rsqrt in squash = bitcast magic-seed + Newton on DVE;
    1/z and 1/(1+n2) via DVE reciprocal.
  - o-reduction of ihat*v: bf16 2x-mode pairwise tree adds.
  - softmax state multiplicative: e *= exp(delta), no f32 bias tensor.
  - zsc coupling lhsT layout (g, b, c), all-bf16 step-1 -> 2x mask mult;
    rz materialized dense by scalar engine so coup mult also runs 2x.
  - einsum PSUM evacuation: 3 ig per 2KB PSUM bank, one scalar copy each.
  - batch tiles software-pipelined: emission order interleaves bt's so
    the DVE stream never waits on PE s-matmuls / scalar exp of the same
    chain; routing state lives in bufs=2 pools, ihat in bufs=3.
"""

import sys

if "/opt/trn_rl_repo" not in sys.path:
    sys.path.insert(0, "/opt/trn_rl_repo")

import numpy as np

B, I, D, C, O = 256, 1152, 8, 10, 16
NCORES = 8
BL = B // NCORES            # 32 local samples per core
NBT, BT = 4, 8              # batch tiles
ISUB = 16                   # i's per group
IG = I // ISUB              # 72 groups
CO = C * O                  # 160
NR = 3
EPS = 1e-7
XZ_CHUNK = 18               # ig's per xz DMA chunk
NCH = 4                     # bias-update chunks
F32 = np.float32
MAGIC = float(0x5F3759DF)   # rsqrt seed magic

USE_BF16 = True

_compiled = {}


def _build_program():
    import concourse.bacc as bacc
    import concourse.tile as tile
    import concourse.mybir as mybir
    import concourse.bass as bass

    f32 = mybir.dt.float32
    i32 = mybir.dt.int32
    lo = mybir.dt.bfloat16 if USE_BF16 else f32
    nc = bacc.Bacc("TRN2", target_bir_lowering=False, debug=False,
                   num_devices=NCORES)

    xz_t = nc.dram_tensor("xz", [NBT * IG, 128, 128], lo, kind="ExternalInput")
    xt_t = nc.dram_tensor("xt", [128, IG, BL], lo, kind="ExternalInput")
    w_t = nc.dram_tensor("w", [128, IG * CO], lo, kind="ExternalInput")
    cmask_t = nc.dram_tensor("cmask", [BT * C, CO], f32, kind="ExternalInput")
    maskz_t = nc.dram_tensor("maskz", [128, BT * C], lo, kind="ExternalInput")
    sel_t = nc.dram_tensor("sel", [BT * C, BT], f32, kind="ExternalInput")
    out_t = nc.dram_tensor("out", [BL, CO], f32, kind="ExternalOutput")
    vscr_t = nc.dram_tensor("vscr", [BL, CO], f32)   # internal scratch
    xz_ap, xt_ap, w_ap = xz_t.ap(), xt_t.ap(), w_t.ap()
    out_ap, vscr_ap = out_t.ap(), vscr_t.ap()

    AF = mybir.ActivationFunctionType
    ALU = mybir.AluOpType
    AX = mybir.AxisListType

    GN = IG // NCH           # 18 groups per chunk
    GC = GN * C              # 180

    with tile.TileContext(nc) as tc:
        from contextlib import ExitStack

        with ExitStack() as ctx:
            singles = ctx.enter_context(tc.tile_pool(name="singles", bufs=1))
            xzp = ctx.enter_context(tc.tile_pool(name="xzp", bufs=3))
            psum = ctx.enter_context(tc.tile_pool(name="psum", bufs=4, space="PSUM"))
            psm = ctx.enter_context(tc.tile_pool(name="psm", bufs=2, space="PSUM"))
            ihp = ctx.enter_context(tc.tile_pool(name="ihp", bufs=3))
            tch = ctx.enter_context(tc.tile_pool(name="tch", bufs=2))
            trp = ctx.enter_context(tc.tile_pool(name="trp", bufs=2))
            dp = ctx.enter_context(tc.tile_pool(name="dp", bufs=2))
            ep = ctx.enter_context(tc.tile_pool(name="ep", bufs=2))
            cp = ctx.enter_context(tc.tile_pool(name="cp", bufs=2))
            zp = ctx.enter_context(tc.tile_pool(name="zp", bufs=2))
            vp = ctx.enter_context(tc.tile_pool(name="vp", bufs=2))
            sm = ctx.enter_context(tc.tile_pool(name="sm", bufs=2))

            w_sb = singles.tile([128, IG * CO], lo)
            nc.sync.dma_start(out=w_sb, in_=w_ap)
            xt_sb = singles.tile([128, IG * BL], lo)
            nc.sync.dma_start(out=xt_sb,
                              in_=xt_ap.rearrange("p g b -> p (g b)"))
            cmask = singles.tile([BT * C, CO], f32)
            nc.sync.dma_start(out=cmask, in_=cmask_t.ap())
            maskz = singles.tile([128, BT * C], lo)
            nc.sync.dma_start(out=maskz, in_=maskz_t.ap())
            sel_sb = singles.tile([BT * C, BT], f32)
            nc.sync.dma_start(out=sel_sb, in_=sel_t.ap())

            def rsqrt_dve(pool, a, p, w, iters):
                """y ~= 1/sqrt(a) on DVE only (magic seed + Newton)."""
                sh = pool.tile([p, w], i32, name="rs_sh", tag="rs_sh")
                nc.vector.tensor_scalar(sh, a.bitcast(i32), 1, None,
                                        op0=ALU.logical_shift_right)
                yi = pool.tile([p, w], i32, name="rs_yi", tag="rs_yi")
                nc.vector.tensor_scalar(yi, sh, -1.0, MAGIC,
                                        op0=ALU.mult, op1=ALU.add)
                y = yi.bitcast(f32)
                for _ in range(iters):
                    t = pool.tile([p, w], f32, name="rs_t", tag="rs_t")
                    nc.vector.tensor_tensor(t, y, y, op=ALU.mult)
                    nc.vector.tensor_tensor(t, t, a, op=ALU.mult)
                    nc.vector.tensor_scalar(t, t, -0.5, 1.5,
                                            op0=ALU.mult, op1=ALU.add)
                    yn = pool.tile([p, w], f32, name="rs_yn", tag="rs_yn")
                    nc.vector.tensor_tensor(yn, y, t, op=ALU.mult)
                    y = yn
                return y

            def squash_scale(pool, n2, p, w, iters):
                """f = n2 / ((1+n2)*sqrt(n2+eps)), DVE only, [p, w]."""
                dn = pool.tile([p, w], f32, name="sq_dn", tag="sq_dn")
                nc.vector.tensor_scalar_add(dn, n2, 1.0)
                wi = pool.tile([p, w], f32, name="sq_wi", tag="sq_wi")
                nc.vector.reciprocal(wi, dn)
                a = pool.tile([p, w], f32, name="sq_a", tag="sq_a")
                nc.vector.tensor_scalar_add(a, n2, EPS)
                y = rsqrt_dve(pool, a, p, w, iters)
                f = pool.tile([p, w], f32, name="sq_f", tag="sq_f")
                nc.vector.tensor_tensor(f, n2, wi, op=ALU.mult)
                nc.vector.tensor_tensor(f, f, y, op=ALU.mult)
                return f

            # ---- r0 weighted sum: s0 = 0.1 * sum_{i,d} x*W  (all 32 b) ----
            ps0 = psm.tile([BL, CO], f32, tag="pss")
            for kc in range(IG):
                nc.tensor.matmul(ps0, xt_sb[:, kc * BL:(kc + 1) * BL],
                                 w_sb[:, kc * CO:(kc + 1) * CO],
                                 start=(kc == 0), stop=(kc == IG - 1))
            s_all = singles.tile([BL, CO], f32)
            nc.scalar.mul(s_all, ps0, 1.0 / C)

            # r0 squash on [32, CO]: per-(b,c) n2 over o, then scale
            sq32 = singles.tile([BL, CO], f32)
            nc.vector.tensor_mul(sq32, s_all, s_all)
            n2_32 = singles.tile([BL, C], f32)
            nc.vector.tensor_reduce(
                n2_32, sq32.rearrange("p (c o) -> p c o", c=C),
                axis=AX.X, op=ALU.add)
            f32t = squash_scale(sm, n2_32, BL, C, iters=2)
            v0 = singles.tile([BL, CO], f32)
            fb = bass.AP(tensor=f32t.tensor, offset=f32t.offset,
                         ap=[f32t.ap[0], f32t.ap[1], [0, O]])
            nc.vector.tensor_tensor(v0, s_all, fb, op=ALU.mult)
            nc.sync.dma_start(out=vscr_ap, in_=v0)

            st = {}  # per-bt pipeline state

            def emit_einsum(bt):
                ihat = ihp.tile([128, IG * CO], lo, name=f"ihat{bt}", tag="ihat")
                for ch in range(IG // XZ_CHUNK):
                    xz_sb = xzp.tile([128, XZ_CHUNK * 128], lo,
                                     name=f"xz{bt}_{ch}", tag="xz")
                    base = bt * IG + ch * XZ_CHUNK
                    nc.sync.dma_start(
                        out=xz_sb.rearrange("p (t m) -> p t m", t=XZ_CHUNK),
                        in_=xz_ap[base:base + XZ_CHUNK].rearrange(
                            "t p m -> p t m"))
                    for t3 in range(XZ_CHUNK // 3):
                        pih = psum.tile([128, 3 * CO], f32,
                                        name=f"pih{bt}_{ch}_{t3}", tag="pih")
                        for j in range(3):
                            t = t3 * 3 + j
                            ig = ch * XZ_CHUNK + t
                            nc.tensor.matmul(
                                pih[:, j * CO:(j + 1) * CO],
                                xz_sb[:, t * 128:(t + 1) * 128],
                                w_sb[:, ig * CO:(ig + 1) * CO],
                                start=True, stop=True)
                        ig0 = ch * XZ_CHUNK + t3 * 3
                        nc.scalar.copy(
                            ihat[:, ig0 * CO:(ig0 + 3) * CO], pih)
                st[bt] = {"ihat": ihat}

            def emit_h1(bt, r):
                s = st[bt]
                ihat = s["ihat"]
                vrep = vp.tile([128, CO], lo, name=f"vrep{bt}_{r}", tag="vrep")
                if r == 0:
                    vi = bass.AP(tensor=vscr_ap.tensor,
                                 offset=bt * BT * CO,
                                 ap=[[CO, BT], [0, ISUB], [1, CO]])
                else:
                    vsrc = s["v"]
                    vi = bass.AP(tensor=vsrc.tensor, offset=vsrc.offset,
                                 ap=[vsrc.ap[0], [0, ISUB], [1, CO]])
                nc.gpsimd.dma_start(out=vrep, in_=vi)

                delta = dp.tile([128, IG * C], f32, name=f"delta{bt}_{r}", tag="delta")
                for ch in range(NCH):
                    g0 = ch * GN
                    tc_t = tch.tile([128, GN * CO], lo, name=f"tc{bt}{r}{ch}", tag="tc")
                    vb = bass.AP(tensor=vrep.tensor, offset=vrep.offset,
                                 ap=[vrep.ap[0], [0, GN], [1, CO]])
                    nc.vector.tensor_tensor(
                        tc_t, ihat[:, g0 * CO:(g0 + GN) * CO], vb,
                        op=ALU.mult)
                    t8 = trp.tile([128, GC * 8], lo, name=f"t8_{bt}{r}{ch}", tag="t8")
                    a0 = bass.AP(tensor=tc_t.tensor, offset=tc_t.offset,
                                 ap=[tc_t.ap[0], [16, GC], [1, 8]])
                    a1 = bass.AP(tensor=tc_t.tensor, offset=tc_t.offset + 8,
                                 ap=[tc_t.ap[0], [16, GC], [1, 8]])
                    d8 = bass.AP(tensor=t8.tensor, offset=t8.offset,
                                 ap=[t8.ap[0], [8, GC], [1, 8]])
                    nc.vector.tensor_tensor(d8, a0, a1, op=ALU.add)
                    t4 = trp.tile([128, GC * 4], lo, name=f"t4_{bt}{r}{ch}", tag="t4")
                    b0 = bass.AP(tensor=t8.tensor, offset=t8.offset,
                                 ap=[t8.ap[0], [8, GC], [1, 4]])
                    b1 = bass.AP(tensor=t8.tensor, offset=t8.offset + 4,
                                 ap=[t8.ap[0], [8, GC], [1, 4]])
                    d4 = bass.AP(tensor=t4.tensor, offset=t4.offset,
                                 ap=[t4.ap[0], [4, GC], [1, 4]])
                    nc.vector.tensor_tensor(d4, b0, b1, op=ALU.add)
                    t2 = trp.tile([128, GC * 2], lo, name=f"t2_{bt}{r}{ch}", tag="t2")
                    c0 = bass.AP(tensor=t4.tensor, offset=t4.offset,
                                 ap=[t4.ap[0], [4, GC], [1, 2]])
                    c1 = bass.AP(tensor=t4.tensor, offset=t4.offset + 2,
                                 ap=[t4.ap[0], [4, GC], [1, 2]])
                    d2 = bass.AP(tensor=t2.tensor, offset=t2.offset,
                                 ap=[t2.ap[0], [2, GC], [1, 2]])
                    nc.vector.tensor_tensor(d2, c0, c1, op=ALU.add)
                    e0 = bass.AP(tensor=t2.tensor, offset=t2.offset,
                                 ap=[t2.ap[0], [2, GC]])
                    e1a = bass.AP(tensor=t2.tensor, offset=t2.offset + 1,
                                  ap=[t2.ap[0], [2, GC]])
                    nc.vector.tensor_tensor(
                        delta[:, ch * GC:(ch + 1) * GC], e0, e1a,
                        op=ALU.add)

                # e = exp(bias), accumulated multiplicatively
                if r == 0:
                    e_t = ep.tile([128, IG * C], lo, name=f"e{bt}", tag="e")
                    nc.scalar.activation(e_t, delta, AF.Exp)
                    s["e"] = e_t
                else:
                    e_t = s["e"]
                    ed = sm.tile([128, IG * C], lo, name=f"ed{bt}", tag="ed")
                    nc.scalar.activation(ed, delta, AF.Exp)
                    nc.vector.tensor_tensor(e_t, e_t, ed, op=ALU.mult)

                zsum = sm.tile([128, IG], f32, name=f"zs{bt}{r}", tag="zs")
                nc.vector.tensor_reduce(
                    zsum, e_t.rearrange("p (g c) -> p g c", c=C),
                    axis=AX.X, op=ALU.add)
                rz = sm.tile([128, IG], f32, name=f"rz{bt}{r}", tag="rz")
                nc.vector.reciprocal(rz, zsum)
                # materialize rz dense (scalar engine) so coup mult is 2x
                rz720 = sm.tile([128, IG * C], lo, name=f"rzm{bt}{r}", tag="rzm")
                rzb = bass.AP(tensor=rz.tensor, offset=rz.offset,
                              ap=[rz.ap[0], [1, IG], [0, C]])
                nc.scalar.copy(rz720, rzb)
                coup = cp.tile([128, IG * C], lo, name=f"coup{bt}{r}", tag="coup")
                nc.vector.tensor_tensor(coup, e_t, rz720, op=ALU.mult)

                # zsc[(b,i),(g,b',c)] = coup[(b,i),(g,c)] * d(b,b')
                zsc = zp.tile([128, IG * BT * C], lo, name=f"zsc{bt}{r}", tag="zsc")
                zr = zsc.rearrange("p (g b c) -> p g b c", b=BT, c=C)
                cb = bass.AP(tensor=coup.tensor, offset=coup.offset,
                             ap=[coup.ap[0], [C, IG], [0, BT], [1, C]])
                mb = bass.AP(tensor=maskz.tensor, offset=maskz.offset,
                             ap=[maskz.ap[0], [0, IG], [C, BT], [1, C]])
                nc.vector.tensor_tensor(zr, cb, mb, op=ALU.mult)
                s["zsc"] = zsc

            def emit_h2(bt, r):
                s = st[bt]
                ihat, zsc = s["ihat"], s["zsc"]
                pss = psm.tile([BT * C, CO], f32, name=f"pss{bt}{r}", tag="pss")
                for ig in range(IG):
                    nc.tensor.matmul(
                        pss, zsc[:, ig * BT * C:(ig + 1) * BT * C],
                        ihat[:, ig * CO:(ig + 1) * CO],
                        start=(ig == 0), stop=(ig == IG - 1))
                sst = sm.tile([BT * C, CO], f32, name=f"sst{bt}{r}", tag="sst")
                nc.vector.tensor_tensor(sst, pss, cmask, op=ALU.mult)
                sjunk = sm.tile([BT * C, CO], f32, name=f"sj{bt}{r}", tag="sj")
                n2_80 = sm.tile([BT * C, 1], f32, name=f"n2{bt}{r}", tag="n2")
                nc.scalar.activation(sjunk, sst, AF.Square,
                                     accum_out=n2_80)
                f80 = squash_scale(sm, n2_80, BT * C, 1,
                                   iters=2 if r == NR - 2 else 1)
                v80 = sm.tile([BT * C, CO], f32, name=f"v80{bt}{r}", tag="v80")
                nc.vector.tensor_scalar_mul(v80, sst, f80)
                v8ps = psm.tile([BT, CO], f32, name=f"v8p{bt}{r}", tag="v8p", bufs=1)
                nc.tensor.matmul(v8ps, sel_sb, v80, start=True, stop=True)
                v_sb = sm.tile([BT, CO], f32, name=f"v{bt}{r}", tag="v")
                nc.scalar.copy(v_sb, v8ps)
                s["v"] = v_sb

            def emit_out(bt):
                nc.sync.dma_start(out=out_ap[bt * BT:(bt + 1) * BT, :],
                                  in_=st[bt]["v"])

            # software-pipelined schedule (see module docstring)
            emit_einsum(0)
            emit_einsum(1)
            emit_h1(0, 0)
            emit_h1(1, 0)
            emit_h2(0, 0)
            emit_h1(0, 1)
            emit_h2(1, 0)
            emit_h2(0, 1)
            emit_out(0)
            emit_einsum(2)
            emit_h1(1, 1)
            emit_h1(2, 0)
            emit_h2(1, 1)
            emit_out(1)
            emit_h2(2, 0)
            emit_einsum(3)
            emit_h1(3, 0)
            emit_h1(2, 1)
            emit_h2(3, 0)
            emit_h2(2, 1)
            emit_out(2)
            emit_h1(3, 1)
            emit_h2(3, 1)
            emit_out(3)

    nc.compile()
    return nc


def _prep_inputs(x, W):
    """Host-side layout transforms (not part of measured HW time)."""
    x = np.ascontiguousarray(x, dtype=F32)
    W = np.ascontiguousarray(W, dtype=F32)
    # W -> [(i_sub, d), (ig, c, o)]
    wr = np.ascontiguousarray(
        W.reshape(IG, ISUB, C, D, O).transpose(1, 3, 0, 2, 4)
    ).reshape(128, IG * CO)

    # x -> per core [core, bt, b, ig, i_sub, d]
    x8 = x.reshape(NCORES, NBT, BT, IG, ISUB, D)

    # block-diagonal lhsT tiles: xz[core, bt, ig, (i_sub,d), (b,i_sub')]
    xz = np.zeros((NCORES, NBT, IG, ISUB, D, 128), dtype=F32)
    isub = np.arange(ISUB)
    for b in range(BT):
        # advanced indexing pulls the i_sub axis to the front
        xz[:, :, :, isub, :, b * ISUB + isub] = \
            x8[:, :, b].transpose(3, 0, 1, 2, 4)
    xz = xz.reshape(NCORES, NBT * IG, 128, 128)

    # compact xT for r0: [core, (i_sub,d), ig, b]
    xt = np.ascontiguousarray(
        x8.reshape(NCORES, BL, IG, ISUB, D).transpose(0, 3, 4, 2, 1)
    ).reshape(NCORES, 128, IG, BL)

    # constants (all (b,c)-ordered partition/row layouts)
    cmask = np.zeros((BT * C, CO), dtype=F32)       # [(b,c'), (c,o)]
    for b in range(BT):
        for c in range(C):
            cmask[b * C + c, c * O:(c + 1) * O] = 1.0
    # maskz[p=(b,i), (b',c)] = 1 iff b' == b
    maskz = np.zeros((128, BT * C), dtype=F32)
    for b in range(BT):
        for c in range(C):
            maskz[b * ISUB:(b + 1) * ISUB, b * C + c] = 1.0
    sel = np.zeros((BT * C, BT), dtype=F32)         # [(b,c'), b2]
    for b in range(BT):
        for c in range(C):
            sel[b * C + c, b] = 1.0

    if USE_BF16:
        from ml_dtypes import bfloat16
        xz = xz.astype(bfloat16)
        xt = xt.astype(bfloat16)
        wr = wr.astype(bfloat16)
        maskz = maskz.astype(bfloat16)
    return xz, xt, wr, cmask, maskz, sel


def kernel(x: np.ndarray, W: np.ndarray) -> np.ndarray:
    from concourse import bass_utils

    if "nc" not in _compiled:
        _compiled["nc"] = _build_program()
    nc = _compiled["nc"]

    xz, xt, wr, cmask, maskz, sel = _prep_inputs(np.asarray(x), np.asarray(W))
    in_maps = [{"xz": xz[c], "xt": xt[c], "w": wr,
                "cmask": cmask, "maskz": maskz, "sel": sel}
               for c in range(NCORES)]
    res = bass_utils.run_bass_kernel_spmd(nc, in_maps, list(range(NCORES)))
    out = np.concatenate([res.results[c]["out"] for c in range(NCORES)], axis=0)
    return out.reshape(B, C, O)
